# revision 69
# baseline (speedup 1.0000x reference)
"""Bass/Trainium2 kernel for nn_Attention (Bahdanau-style attention scores).

reference:
    h = hidden[0]                               # (B, H)
    e = encoder_outputs.swapaxes(0, 1)          # (B, S, H)
    energy = tanh(e @ We.T + h @ Wh.T + b)      # (B, S, H)
    scores = energy @ v                         # (B, S)
    out = softmax(scores, axis=1)[:, None, :]   # (B, 1, S)

Strategy: data-parallel over batch B=32 across 8 cores (4 batches/core,
no collectives). Per core, layout [k partitions, s free]:
  - main matmul in fp8(e4m3) with perf_mode=DoubleRow: the PE array holds
    2 fp8 weights per cell, so one matmul contracts K=256 (an h-pair) and
    the 1024-deep contraction takes 4 matmuls instead of 8 — ~1.8x the
    bf16 matmul throughput. Inputs are pre-scaled (We*64, e*16, both
    powers of two, far from the fp8e4 +-240 range limit) and the 1/1024
    rescale is fused into the ACT-engine tanh via its scale operand.
  - bias (h @ Wh.T + b) is computed once per core (bf16 matmuls) as
    per-partition column vectors and fused into the tanh.
  - the v-dot runs on the DVE as a per-k-tile weighted accumulate
    (acc = energy * v[kt] + acc), finished by one K=128, M=1 ones-matmul
    partition-reduce per chunk, so the PE only runs the main matmuls.
  - softmax over S without max-subtraction (scores are O(1), exp is safe in
    fp32): chunked exp straight out of PSUM with fused accumulate, combine
    sums, reciprocal, chunked scale.
Host side pre-transposes W/encoder_outputs so all device DMAs are
coalesced; output is fp32.
"""
import numpy as np

S, B, H = 2048, 32, 1024
NCORES = 8
BPC = B // NCORES           # batches per core = 4
KT = H // 128               # 8 k-tiles (output dim of We)
HT = H // 128               # 8 h-tiles (contraction dim)
PT = HT // 2                # 4 h-pairs (DoubleRow contracts 256 per matmul)
HB = H + 128                # 1152 = padded contraction for [Wh | b] with ones row
HBT = HB // 128             # 9
NSC = 4                     # s-chunks per batch
SC = S // NSC               # 512
S2 = S // 2                 # 1024 = s-half per e DMA tile
SE = 16.0                   # fp8 scale for e
SW = 64.0                   # fp8 scale for We

_cache = {}


def _build():
    import concourse.tile as tile
    from concourse import bacc, mybir

    f32 = mybir.dt.float32
    bf16 = mybir.dt.bfloat16
    f16 = mybir.dt.float16
    f8 = mybir.dt.float8e4
    DR = mybir.MatmulPerfMode.DoubleRow
    Tanh = mybir.ActivationFunctionType.Tanh
    Exp = mybir.ActivationFunctionType.Exp
    Copy = mybir.ActivationFunctionType.Copy

    nc = bacc.Bacc("TRN2", target_bir_lowering=False, debug=False,
                   num_devices=NCORES)

    # e packed host-side as [b, pair, s-half, p, i, s] so ONE coalesced DMA
    # loads a [128, 2, S/2] DoubleRow half-pair tile (DMA instruction issue
    # costs ~0.65us serialized on the issuing sequencer, so few+big is key;
    # halves rather than full pairs so the first matmuls start ~1us sooner)
    eT_d = nc.dram_tensor("eT", [BPC, PT, 2, 128, 2, S2], f8,
                          kind="ExternalInput").ap()
    # weights packed host-side per kt-column so each loads with one coalesced
    # DMA: WeTp[kt][p, ht*128+j] = WeT[ht*128+p, kt*128+j] (fp8, pre-scaled),
    # WhbTp likewise in bf16 for the bias path
    WeT_d = nc.dram_tensor("WeTp", [KT, 128, H], f8, kind="ExternalInput").ap()
    WhbT_d = nc.dram_tensor("WhbTp", [KT, 128, HB], bf16,
                            kind="ExternalInput").ap()
    # hT/v packed host-side as [128, n] so each loads with one coalesced DMA
    # (the naive [128, BPC]/[128, 1] scatters cost ~10us of 8-byte descriptors)
    hT_d = nc.dram_tensor("hTp", [128, HBT * BPC], bf16,
                          kind="ExternalInput").ap()
    # v plus a trailing all-ones column (used for the partition-reduce matmul)
    v_d = nc.dram_tensor("vp", [128, KT + 1], bf16, kind="ExternalInput").ap()
    out_d = nc.dram_tensor("out", [BPC, S], f32, kind="ExternalOutput").ap()

    with tile.TileContext(nc) as tc:
        with (
            tc.tile_pool(name="w", bufs=1) as wpool,
            tc.tile_pool(name="e", bufs=4 * PT) as epool,
            tc.tile_pool(name="en", bufs=3) as enpool,
            tc.tile_pool(name="acc", bufs=2) as apool,
            tc.tile_pool(name="sm", bufs=2) as spool,
            tc.tile_pool(name="pp", bufs=6, space="PSUM") as ppool,
            tc.tile_pool(name="pv", bufs=2, space="PSUM") as pvpool,
        ):
            # startup DMA order on sync's single HWDGE FIFO (one instruction
            # issues every ~0.65us; transfers complete in issue order at full
            # HBM bandwidth): what the first matmuls need comes first —
            # WeT[0], then the four e[b=0] pair-tiles, then everything else
            # at its consumption pace
            # PE warm-up: the HAM clock gate keeps the PE at 1.2 GHz until
            # ~3.4us of sustained activity. Run dummy matmuls on a zeroed
            # tile while the first DMAs are still in flight so the real
            # matmul stream starts at the full 2.4 GHz
            warm = wpool.tile([128, SC], bf16, tag="warm")
            nc.vector.memset(warm[:], 0)
            for _ in range(12):
                wps = ppool.tile([128, SC], f32, tag="mp", name="warm_ps")
                nc.tensor.matmul(wps[:], lhsT=warm[:, :128], rhs=warm[:],
                                 start=True, stop=True)

            WeT_sb = [None] * KT
            t = wpool.tile([128, HT, 128], f8, tag="WeT0", name="WeT_t")
            nc.sync.dma_start(t[:], WeT_d[0])
            WeT_sb[0] = t
            e_sb0 = []
            for pt in range(PT):
                hts = []
                for hf in range(2):
                    t = epool.tile([128, 2, S2], f8, tag="e", name="e_t")
                    nc.sync.dma_start(t[:], eT_d[0, pt, hf])
                    hts.append(t)
                e_sb0.append(hts)
            WeT_sb[1] = wpool.tile([128, HT, 128], f8, tag="WeT1",
                                   name="WeT_t")
            nc.sync.dma_start(WeT_sb[1][:], WeT_d[1])
            WhbT_sb = [wpool.tile([128, HB], bf16, tag="WhbT0",
                                  name="WhbT_t")]
            nc.sync.dma_start(WhbT_sb[0][:], WhbT_d[0])
            hT_p = wpool.tile([128, HBT * BPC], bf16, tag="hTp")
            nc.sync.dma_start(hT_p[:], hT_d[:])
            hT_sb = [hT_p[:, ht * BPC:(ht + 1) * BPC] for ht in range(HBT)]
            v_p = wpool.tile([128, KT + 1], bf16, tag="vp_sb")
            nc.sync.dma_start(v_p[:], v_d[:])
            v_sb = [v_p[:, kt:kt + 1] for kt in range(KT)]
            ones_sb = v_p[:, KT:KT + 1]
            for kt in range(1, KT):
                if kt > 1:
                    t = wpool.tile([128, HT, 128], f8, tag=f"WeT{kt}",
                                   name="WeT_t")
                    nc.sync.dma_start(t[:], WeT_d[kt])
                    WeT_sb[kt] = t
                t = wpool.tile([128, HB], bf16, tag=f"WhbT{kt}", name="WhbT_t")
                nc.sync.dma_start(t[:], WhbT_d[kt])
                WhbT_sb.append(t)

            bias_sb = wpool.tile([128, KT * BPC], f32, tag="bias")

            def emit_tail(b, accs, rev=False):
                # partition-reduce of the weighted energies: scores[s] =
                # ones.T @ acc (K=128, M=1). Two s-chunks per PSUM bank at
                # col-groups 0/32 (concurrent matmuls, no bank rotation
                # against the exp reads, and exp(sc0/1) can start while the
                # second bank's matmuls still run)
                # two s-chunks per PSUM bank at col-groups 0/32 (concurrent
                # matmuls; col-group 96 NaNs on HW — PE quadrant-3 bug — so
                # never pack more than two)
                sl = [None] * NSC
                for half in ([1, 0] if rev else [0, 1]):
                    vp_t = pvpool.tile([128, SC], f32, tag="vp",
                                       name="vp_t")
                    for j in ([1, 0] if rev else [0, 1]):
                        sc = 2 * half + j
                        nc.tensor.matmul(vp_t[32 * j:32 * j + 1, :],
                                         lhsT=ones_sb,
                                         rhs=accs[:, sc * SC:
                                                  (sc + 1) * SC],
                                         start=True, stop=True,
                                         tile_position=(0, 32 * j))
                        sl[sc] = vp_t[32 * j:32 * j + 1, :]
                # softmax over S (no max subtraction; scores are O(1))
                ex = spool.tile([1, S], f32, tag="exp")
                ssums = spool.tile([1, NSC], f32, tag="ssums")
                for sc in (range(NSC - 1, -1, -1) if rev else range(NSC)):
                    if rev and sc != 0:
                        # last batch: sum these chunks on the (idle) DVE so
                        # no READ_ACCUMULATOR sits in the ACT FIFO between
                        # the exps -- the chain-closing exp(sc0) starts
                        # ~0.66us sooner
                        nc.scalar.activation(ex[:, sc * SC:(sc + 1) * SC],
                                             sl[sc], Exp)
                        nc.vector.tensor_reduce(
                            ssums[:, sc:sc + 1],
                            ex[:, sc * SC:(sc + 1) * SC],
                            axis=mybir.AxisListType.X,
                            op=mybir.AluOpType.add)
                    else:
                        nc.scalar.activation(ex[:, sc * SC:(sc + 1) * SC],
                                             sl[sc], Exp,
                                             accum_out=ssums[:, sc:sc + 1])
                stot = spool.tile([1, 1], f32, tag="stot")
                nc.vector.tensor_reduce(stot[:], ssums[:],
                                        axis=mybir.AxisListType.X,
                                        op=mybir.AluOpType.add)
                rec = spool.tile([1, 1], f32, tag="rec")
                nc.vector.reciprocal(rec[:], stot[:])
                ot = spool.tile([1, S], f32, tag="ot")
                # the two normalization scales run concurrently on DVE/ACT,
                # each half's store DMA issues as soon as that half is done
                nc.vector.tensor_scalar_mul(ot[:, :2 * SC], ex[:, :2 * SC],
                                            rec[:])
                nc.scalar.mul(ot[:, 2 * SC:], ex[:, 2 * SC:], rec[:])
                # the two store DMAs issue from DIFFERENT HWDGE engines
                # (sync + scalar) so their ~0.65us issue costs overlap
                nc.sync.dma_start(out_d[b:b + 1, :2 * SC], ot[:, :2 * SC])
                nc.scalar.dma_start(out_d[b:b + 1, 2 * SC:], ot[:, 2 * SC:])

            # ---- main loop over batches ----
            prev_accs = None
            for b in range(BPC):
                if b == 0:
                    e_sb = e_sb0
                else:
                    # b=1 rides the sync HWDGE FIFO (behind the startup
                    # weights, so it never competes with b=0's critical
                    # loads); b>=2 goes through gpsimd's SWDGE at leisure
                    eng = nc.sync if b == 1 else nc.gpsimd
                    e_sb = []
                    for pt in range(PT):
                        hts = []
                        for hf in range(2):
                            t = epool.tile([128, 2, S2], f8, tag="e",
                                           name="e_t")
                            eng.dma_start(t[:], eT_d[b, pt, hf])
                            hts.append(t)
                        e_sb.append(hts)
                accs = None
                for kt in range(KT):
                    mps = [ppool.tile([128, SC], f32, tag="mp", name="mps")
                           for _ in range(NSC)]
                    # s-halves outer: each accumulation group closes half a
                    # block early, so its tanh can drain the PSUM bank
                    # before the NEXT block's matmuls need it (the weight
                    # tile reloads per half keep the LDW pipe at ~50%).
                    # The very last block closes its groups in REVERSE order
                    # (sc3 first) so the end-of-kernel chain hangs off sc0
                    # with the other three chunks' tanh/acc/exp already done
                    last = (b == BPC - 1 and kt == KT - 1)
                    sc_order = list(range(NSC - 1, -1, -1)) if last \
                        else list(range(NSC))
                    for half in ([1, 0] if last else [0, 1]):
                        for pt in range(PT):
                            for j in ([1, 0] if last else [0, 1]):
                                sc = 2 * half + j
                                et = e_sb[pt][half]
                                of = j * SC
                                nc.tensor.matmul(
                                    mps[sc][:],
                                    lhsT=WeT_sb[kt][:, 2 * pt:2 * pt + 2, :],
                                    rhs=et[:, :, of:of + SC],
                                    start=(pt == 0), stop=(pt == PT - 1),
                                    perf_mode=DR,
                                )
                    if b == 0:
                        # bias = hidden @ Wh.T + b for kt-column kt, emitted
                        # after the matching main block so the PE stream
                        # consumes WhbT tiles at their (later) arrival pace.
                        # Its PSUM rides the v-dot pool (idle during blocks)
                        # so the main pool keeps all its spare banks
                        ph = pvpool.tile([128, SC], f32, tag="vp", name="ph")
                        for ht in range(HBT):
                            nc.tensor.matmul(
                                ph[:, :BPC],
                                lhsT=WhbT_sb[kt][:, ht * 128:(ht + 1) * 128],
                                rhs=hT_sb[ht][:],
                                start=(ht == 0), stop=(ht == HBT - 1),
                            )
                        # copy on ACT (not DVE): the DVE's long merged
                        # accumulate ops would delay this tiny copy and with
                        # it the PSUM-bank handoff to the next matmul block
                        nc.scalar.activation(
                            bias_sb[:, kt * BPC:(kt + 1) * BPC],
                            ph[:, :BPC], Copy)
                    en_t = enpool.tile([128, S], bf16, tag="en", name="en")
                    for sc in sc_order:
                        # fold the fp8 pre-scales out: tanh(psum/1024 + bias)
                        nc.scalar.activation(en_t[:, sc * SC:(sc + 1) * SC],
                                             mps[sc][:], Tanh,
                                             bias=bias_sb[:, kt * BPC + b:
                                                          kt * BPC + b + 1],
                                             scale=float(1.0 / (SE * SW)))
                    # weighted partition accumulate on DVE (keeps the v-dot
                    # off the PE): acc = en * v[kt] + acc. One full-S op per
                    # kt instead of four chunked ones: the ~320ns per-op DVE
                    # overhead is the dominant cost at [128,512]. fp16
                    # accumulator rounding (~5e-4 rel) is far below the fp8
                    # matmul quantization error
                    nacc = apool.tile([128, S], f16, tag="acc", name="acc",
                                      bufs=3)
                    if kt == 0:
                        nc.vector.scalar_tensor_tensor(
                            nacc[:], en_t[:], v_sb[kt], en_t[:],
                            op0=mybir.AluOpType.mult,
                            op1=mybir.AluOpType.bypass)
                    elif kt == KT - 1:
                        # last kt chunked per s-quarter so the batch tail
                        # (v-reduce -> exp) pipelines behind each tanh
                        # instead of waiting for one full-S accumulate.
                        # The chunks stay on the DVE: offloading alternate
                        # chunks to GPSIMD (as mul+add pairs) measured FAR
                        # slower -- its software tensor ops run well below
                        # roofline and block its SWDGE DMA descriptor work,
                        # starving the later batches' e loads
                        for sc in sc_order:
                            sl = slice(sc * SC, (sc + 1) * SC)
                            nc.vector.scalar_tensor_tensor(
                                nacc[:, sl], en_t[:, sl], v_sb[kt],
                                accs[:, sl],
                                op0=mybir.AluOpType.mult,
                                op1=mybir.AluOpType.add)
                    else:
                        nc.vector.scalar_tensor_tensor(
                            nacc[:], en_t[:], v_sb[kt], accs[:],
                            op0=mybir.AluOpType.mult,
                            op1=mybir.AluOpType.add)
                    accs = nacc
                    if kt == 1 and prev_accs is not None:
                        # emit the previous batch's v-reduce + softmax AFTER
                        # this batch's first two matmul blocks: by the time
                        # the PE FIFO reaches the v-reduce matmuls, the
                        # tanh->DVE chain of the previous batch has long
                        # finished, so the PE never stalls at batch borders
                        emit_tail(b - 1, prev_accs)
                prev_accs = accs
            emit_tail(BPC - 1, prev_accs, rev=True)

    nc.compile()
    return nc


def _prep_inputs(hidden, encoder_outputs, W, b, v):
    import ml_dtypes
    bf16 = ml_dtypes.bfloat16
    f8 = ml_dtypes.float8_e4m3

    hidden = np.asarray(hidden, dtype=np.float32)
    encoder_outputs = np.asarray(encoder_outputs, dtype=np.float32)
    W = np.asarray(W, dtype=np.float32)
    b = np.asarray(b, dtype=np.float32)
    v = np.asarray(v, dtype=np.float32)

    # (S, B, H) -> (B, H, S) scaled fp8, then packed to
    # [B, PT, 2, 128, 2, S2] so each DoubleRow half-pair tile is ONE DMA
    eT_all = (np.ascontiguousarray(encoder_outputs.transpose(1, 2, 0))
              * np.float32(SE)).astype(f8)
    eT_all = np.ascontiguousarray(
        eT_all.reshape(B, PT, 2, 128, 2, S2).transpose(0, 1, 4, 3, 2, 5))
    WhT = np.ascontiguousarray(W.astype(bf16)[:, :H].T)          # [h, k]
    WeT8 = np.ascontiguousarray((W[:, H:].T * np.float32(SW)).astype(f8))
    WhbT = np.concatenate(
        [WhT, b.astype(bf16)[None, :], np.zeros((127, H), dtype=bf16)], axis=0)
    # pack per kt-column: Xp[kt, p, ht*128+j] = X[ht*128+p, kt*128+j]
    WeTp = np.ascontiguousarray(
        WeT8.reshape(HT, 128, KT, 128).transpose(2, 1, 0, 3).reshape(KT, 128, H))
    WhbTp = np.ascontiguousarray(
        WhbT.reshape(HBT, 128, KT, 128).transpose(2, 1, 0, 3).reshape(KT, 128, HB))
    h_bf = hidden[0].astype(bf16)                      # (B, H)
    # v packed as [128, KT+1]: column kt holds v[kt*128:(kt+1)*128]; the
    # last column is all-ones (stationary vector for the partition-reduce)
    v_p = np.concatenate(
        [v.astype(bf16).reshape(KT, 128).T, np.ones((128, 1), dtype=bf16)],
        axis=1)
    v_p = np.ascontiguousarray(v_p)

    in_maps = []
    for i in range(NCORES):
        sl = slice(i * BPC, (i + 1) * BPC)
        hT = np.concatenate(
            [np.ascontiguousarray(h_bf[sl].T),
             np.ones((1, BPC), dtype=bf16),
             np.zeros((127, BPC), dtype=bf16)], axis=0)    # (HB, BPC)
        # packed as [128, HBT*BPC]: block ht = hT[ht*128:(ht+1)*128, :]
        hT_p = np.ascontiguousarray(
            hT.reshape(HBT, 128, BPC).transpose(1, 0, 2).reshape(128, HBT * BPC))
        in_maps.append({
            "eT": eT_all[sl],
            "WeTp": WeTp,
            "WhbTp": WhbTp,
            "hTp": hT_p,
            "vp": v_p,
        })
    return in_maps


def _install_ntff_hook():
    """Make `antenv.axon_hooks` importable (absent in this image) so that
    run_bass_kernel_spmd(trace=True) / BASS_TRACE=1 works instead of
    crashing on import; profiling hook wired via the axon .so when present."""
    import sys, types
    try:
        import antenv
    except ImportError:
        return
    if "antenv.axon_hooks" in sys.modules:
        return
    mod = types.ModuleType("antenv.axon_hooks")
    state = {"hook": None}
    mod.set_axon_ntff_profile_hook = lambda h: state.__setitem__("hook", h)
    mod.get_axon_ntff_profile_hook = lambda: state["hook"]
    sys.modules["antenv.axon_hooks"] = mod
    antenv.axon_hooks = mod
    try:
        from trn_agent_boot.trn_boot import _ntff_profile_via_ctypes
        mod.set_axon_ntff_profile_hook(
            _ntff_profile_via_ctypes("/opt/axon/libaxon_pjrt.so"))
    except Exception:
        pass


def kernel_with_results(hidden, encoder_outputs, W, b, v):
    from concourse.bass_utils import run_bass_kernel_spmd

    _install_ntff_hook()
    if "nc" not in _cache:
        _cache["nc"] = _build()
    nc = _cache["nc"]
    in_maps = _prep_inputs(hidden, encoder_outputs, W, b, v)
    res = run_bass_kernel_spmd(nc, in_maps, core_ids=list(range(NCORES)))
    out = np.concatenate([res.results[i]["out"] for i in range(NCORES)], axis=0)
    return out[:, None, :].astype(np.float32), res


def kernel(hidden, encoder_outputs, W, b, v):
    out, _ = kernel_with_results(hidden, encoder_outputs, W, b, v)
    return out
